# revision 32
# baseline (speedup 1.0000x reference)
"""Multi-head attention (B=4, N=2048, D=1024, H=16) on 8 Trainium2 cores — v6.

Sharding: core = (batch b, head-group hg) -> 4 batches x 2 groups of 8 heads.

v6 over the v2 baseline (309us): a PE-sequencer diet, dual-engine exp, and
fp8 DoubleRow on the output projection (the one matmul where e4m3
quantization is NOT amplified by peaked-softmax queries — fp8 anywhere in
the Q/K path measures ~2e-2 max-rel-err by itself and fails the gate).
  - Denominators folded into PV: V tiles carry a 65th ones-column, each PV
    matmul emits [128 qtok, 64 O-feats + 1 denom] — no separate denominator
    matmuls (v2 had 2048 of them; PE.SEQ-bound at ~44ns/matmul dispatch).
  - V projection pair-merged: one [128,128] matmul per (pair, keytok, k)
    covers 2 heads (1024 -> 512 instructions).
  - Output projection in fp8e4m3 DoubleRow (0.5 cyc/row, 2x contraction):
    O scaled x64 into fp8 during the normalize (the x64 rides the
    reciprocal multiply); W_proj scaled x64 host-side (0.02-scale weights
    are e4m3-denormals otherwise); the y-copy descales by 1/4096.
    65.5K -> 16.4K PE cycles, transposes in fp8 at 1.0 cyc/row.
  - Exp split: ACT exact Exp on 11/16 tiles; DVE the rest via a Schraudolph
    tensor_scalar (s*A + B -> int16 whose bits are bf16 exp(s/8); ±4% max,
    ~2.3% rms, zero-mean; softmax denominator uses the same values so the
    bias cancels). Measured numpy-model error of the full config ~9e-3.
  - PSUM zero-region machinery replaced by start=True on each group's first
    write; K/Q/V/proj psum->sbuf copies deferred behind the next exp and
    split across ACT/DVE to fill exp-stream bubbles.
PSUM (8 banks): s 2x[128,1024]=4, o_a 1, o_b 1, y 2 (fillers, rotating).
"""

import sys

if "/opt/trn_rl_repo" not in sys.path:
    sys.path.insert(0, "/opt/trn_rl_repo")

from collections import deque
from contextlib import ExitStack

import numpy as np

B, N, D, H = 4, 2048, 1024, 16
HG = 2                 # head groups (tensor parallel)
NCORES = B * HG        # 8
DH = D // HG           # 512 features per group = 8 heads * 64
HH = H // HG           # 8 heads per core
P = 128
KC = D // P            # 8 contraction chunks over d_model
CP = HH // 2           # 4 head pairs per core
TJ = N // P            # 16 key 128-chunks
IB = 1024              # i-block (exp free-dim)
NI = N // IB           # 2
SCALE = (D // H) ** -0.5
OSCALE = 64.0          # fp8 O/W_proj pre-scale (avoids e4m3 denormals)

LOG2E = 1.4426950408889634
TRICK_A = 128.0 * LOG2E * SCALE
TRICK_B = 16256.0 - 7.3348 + 0.5
DVE_EXP_J = frozenset((2, 5, 8, 11, 14))   # exp tiles routed to DVE

_cached = {}


def _build():
    import concourse.mybir as mybir
    import concourse.tile as tile
    from concourse import bacc, masks

    f32 = mybir.dt.float32
    bf16 = mybir.dt.bfloat16
    fp8 = mybir.dt.float8e4
    i16 = mybir.dt.int16
    AF = mybir.ActivationFunctionType
    DR = mybir.MatmulPerfMode.DoubleRow
    MUL = mybir.AluOpType.mult
    ADD = mybir.AluOpType.add

    nc = bacc.Bacc("TRN2", target_bir_lowering=False, debug=False,
                   enable_asserts=False)

    # host-packed, partition-major DRAM inputs (contiguous per-partition runs)
    xt = nc.dram_tensor("xt", (P, KC, N), bf16, kind="ExternalInput").ap()
    wqh = nc.dram_tensor("wqh", (P, 4, KC, P), bf16,
                         kind="ExternalInput").ap()
    wkh = nc.dram_tensor("wkh", (P, 4, KC, P), bf16,
                         kind="ExternalInput").ap()
    wvh = nc.dram_tensor("wvh", (P, KC, HH * 64), bf16,
                         kind="ExternalInput").ap()
    wp8 = nc.dram_tensor("wp8", (P, 2, 2, D), fp8, kind="ExternalInput").ap()
    bqh = nc.dram_tensor("bqh", (P, 8), f32, kind="ExternalInput").ap()
    y = nc.dram_tensor("y", (N, D), bf16, kind="ExternalOutput").ap()

    with tile.TileContext(nc) as tc, ExitStack() as ctx:
        const = ctx.enter_context(tc.tile_pool(name="const", bufs=1))
        persist = ctx.enter_context(tc.tile_pool(name="persist", bufs=1))
        ppool = ctx.enter_context(tc.tile_pool(name="pp", bufs=8))
        ospool = ctx.enter_context(tc.tile_pool(name="osb", bufs=2))
        otpool = ctx.enter_context(tc.tile_pool(name="ot", bufs=2))
        dpool = ctx.enter_context(tc.tile_pool(name="dv", bufs=2))
        ypool = ctx.enter_context(tc.tile_pool(name="yb", bufs=5))
        psp = ctx.enter_context(tc.tile_pool(name="psp", bufs=1, space="PSUM"))

        # ---- consts ----
        ones_bf = const.tile([P, 16], bf16)
        nc.vector.memset(ones_bf[:], 1.0)

        # ---- persistent SBUF ----
        xt_sb = persist.tile([P, KC, N], bf16)
        wq_sb = persist.tile([P, 4, KC, P], bf16)
        wk_sb = persist.tile([P, 4, KC, P], bf16)
        wv_sb = persist.tile([P, KC, HH * 64], bf16)
        wp_sb = persist.tile([P, 2, 2, D], fp8)
        # K^T/Q^T bf16 [64*b+f64, m, tok]: chunk m = 2*quad + h4//2 holds
        # heads h4 = 2*(m%2) + b at partition blocks b*64 (bases 0/64 only).
        kt2 = persist.tile([P, 4, N], bf16)
        qt2 = persist.tile([P, 4, N], bf16)
        # V [keytok128, j, h, 0:64] + ones in col 64 (PV denominator source)
        vsb = persist.tile([P, TJ, HH, 65], bf16)
        bqk_sb = const.tile([P, 1, 8], f32)             # [q, 1, (t)*4+m]

        # ---- DMAs (prefix-critical first; unit (0,0) needs only m=0) ----
        nc.sync.dma_start(bqk_sb[:, 0, :], bqh)
        nc.sync.dma_start(wk_sb[:, 0:1], wkh[:, 0:1])
        nc.sync.dma_start(xt_sb[:, :, 0:512], xt[:, :, 0:512])
        nc.sync.dma_start(wq_sb[:, 0:1], wqh[:, 0:1])
        nc.sync.dma_start(xt_sb[:, :, 512:1024], xt[:, :, 512:1024])
        nc.sync.dma_start(wv_sb[:, :, 0:128], wvh[:, :, 0:128])
        nc.sync.dma_start(wk_sb[:, 1:4], wkh[:, 1:4])
        nc.sync.dma_start(wq_sb[:, 1:4], wqh[:, 1:4])
        nc.sync.dma_start(wv_sb[:, :, 128:512], wvh[:, :, 128:512])
        nc.sync.dma_start(xt_sb[:, :, 1024:1536], xt[:, :, 1024:1536])
        nc.sync.dma_start(xt_sb[:, :, 1536:2048], xt[:, :, 1536:2048])
        nc.sync.dma_start(wp_sb[:], wp8)

        # vsb's ones column on the idle gpsimd engine before first PV use
        nc.gpsimd.memset(vsb[:, :, :, 64:65], 1.0)

        # identity (fp8: 1.0 cyc/row transposes, exact for 0/1)
        ident_f8 = const.tile([P, P], fp8)
        masks.make_identity(nc, ident_f8[:])

        # preload the exp table while ACT is idle
        dummy = const.tile([1, 16], f32)
        nc.scalar.activation(dummy[:], ones_bf[0:1, :], AF.Exp)

        # warmup: absorb the PE p-state ramp under the initial DMA wait
        wconst = const.tile([P, 512], bf16)
        nc.vector.memset(wconst[:], 0.0)
        for _ in range(12):
            wm = psp.tile([P, IB], f32, tag="s", bufs=2, name="wm")
            nc.tensor.matmul(wm[:, 0:512], wconst[:, 0:P], wconst[:],
                             start=True, stop=True, skip_group_check=True)

        # ================= emission helpers =================
        def _bufs(tag):
            return 2 if tag in ("s", "y") else 1

        # Filler PSUM->SBUF copies are deferred until after the next step's
        # exp dispatch so they never delay a DVE trick-exp in the in-order
        # DVE queue.
        deferred_copies = []

        def emit_k_chunk(m, n, tag):
            """kt2[:, m, n*512:...] — 8 bf16 matmuls + bias-add copy."""
            c0 = n * 512
            pt = psp.tile([P, 512], f32, tag=tag, bufs=_bufs(tag), name="ptk")
            for k in range(KC):
                nc.tensor.matmul(pt[:], wk_sb[:, m, k], xt_sb[:, k, c0:c0 + 512],
                                 start=(k == 0), stop=(k == KC - 1))
            deferred_copies.append(
                lambda m=m, c0=c0, pt=pt: nc.vector.tensor_scalar_add(
                    kt2[:, m, c0:c0 + 512], pt[:],
                    bqk_sb[:, 0, 4 + m:5 + m]))

        def emit_q_chunk(m, n, tag):
            c0 = n * 512
            pt = psp.tile([P, 512], f32, tag=tag, bufs=_bufs(tag), name="ptq")
            for k in range(KC):
                nc.tensor.matmul(pt[:], wq_sb[:, m, k], xt_sb[:, k, c0:c0 + 512],
                                 start=(k == 0), stop=(k == KC - 1))
            deferred_copies.append(
                lambda m=m, c0=c0, pt=pt: nc.vector.tensor_scalar_add(
                    qt2[:, m, c0:c0 + 512], pt[:],
                    bqk_sb[:, 0, m:m + 1]))

        def emit_v_slice(c, g, tag):
            """vsb[:, g, 2c:2c+2, 0:64] — V for head pair c, keytok chunk g,
            both heads in one [128,128] matmul per k-chunk."""
            pv = psp.tile([P, 512], f32, tag=tag, bufs=_bufs(tag), name="ptv")
            for k in range(KC):
                nc.tensor.matmul(pv[:, 0:128],
                                 xt_sb[:, k, g * P:(g + 1) * P],
                                 wv_sb[:, k, c * 128:(c + 1) * 128],
                                 start=(k == 0), stop=(k == KC - 1))

            def vcopy(c=c, g=g, pv=pv):
                # split V copies between ACT (fills exp bubbles) and DVE
                dst = vsb[:, g, 2 * c:2 * c + 2, 0:64]
                src = pv[:, 0:128].rearrange("p (a b) -> p a b", a=2)
                if (c + g) % 2 == 0:
                    nc.scalar.activation(dst, src, AF.Copy)
                else:
                    nc.vector.tensor_copy(dst, src)
            deferred_copies.append(vcopy)

        ot_tiles = {}

        def emit_proj(i, t, o, tag):
            """y chunk via fp8 DoubleRow: contraction 512 = 2 DR chunks of
            [128 part, 2 slot]; ot slots are head-pairs (c, c+... slot i ->
            feat i*128+p within chunk)."""
            ot_i = ot_tiles[i]
            yp = psp.tile([P, 512], f32, tag=tag, bufs=_bufs(tag), name="yp")
            for ch in range(2):
                nc.tensor.matmul(
                    yp[:], ot_i[:, 2 * ch:2 * ch + 2, t * P:(t + 1) * P],
                    wp_sb[:, :, ch, o * 512:(o + 1) * 512]
                    .rearrange("p a b -> p a b"),
                    start=(ch == 0), stop=(ch == 1), perf_mode=DR)

            def ycopy(i=i, t=t, o=o, yp=yp):
                ysb = ypool.tile([P, 512], bf16, tag="ysb", name="ysb")
                nc.vector.tensor_scalar_mul(ysb[:], yp[:],
                                            1.0 / (OSCALE * OSCALE))
                r0 = i * IB + t * P
                nc.sync.dma_start(y[r0:r0 + P, o * 512:(o + 1) * 512],
                                  ysb[:])
            deferred_copies.append(ycopy)

        def flush_copies():
            for fn in deferred_copies:
                fn()
            deferred_copies.clear()

        # filler queue: (deadline_step, pe_cycles, fn(tag)) in deadline order
        fillers = deque()

        def F(cycles, fn, deadline=10**9):
            fillers.append((deadline, cycles, fn))

        def pump(credit, step=-1):
            while fillers and (fillers[0][0] <= step
                               or fillers[0][1] <= credit):
                _, cyc, fn = fillers.popleft()
                fn("y")
                credit -= cyc
            return credit

        # ================= prefix (unit (0,0)=head hh0 uses only m=0) ======
        emit_k_chunk(0, 0, "s")
        emit_q_chunk(0, 0, "s")
        emit_q_chunk(0, 1, "s")
        emit_v_slice(0, 0, "s")
        emit_v_slice(0, 1, "s")
        flush_copies()

        # unit order (i, hh): i1 units of a pair consume no new K/V fillers
        NU = NI * HH                      # 16 units
        seq = [(0, 0), (0, 1), (1, 0), (1, 1),
               (0, 2), (0, 3), (1, 2), (1, 3),
               (0, 4), (0, 5), (0, 6), (0, 7),
               (1, 4), (1, 5), (1, 6), (1, 7)]
        pos_of = {u: p for p, u in enumerate(seq)}

        # first unit position consuming K/Q chunk m (heads hh = 2m, 2m+1)
        def mpos(blk, m):
            return pos_of[(blk, 2 * m)]

        # ---- filler events (deadline-sorted) ----
        events = []   # (deadline_step, pe_cycles, fn)
        for c in range(CP):
            p0 = 16 * pos_of[(0, 2 * c)]
            for g in range(TJ):
                if c == 0 and g < 2:
                    continue   # prefix
                events.append((p0 + g, 1024,
                               lambda tag, c=c, g=g: emit_v_slice(c, g, tag)))
        for m in range(4):
            pk = 16 * mpos(0, m)
            for n in range(4):
                if m == 0 and n == 0:
                    continue   # prefix
                events.append((max(0, pk + 4 * n - 3), 4096,
                               lambda tag, m=m, n=n: emit_k_chunk(m, n, tag)))
            for n in range(4):
                if m == 0 and n < 2:
                    continue   # prefix
                dl = 16 * mpos(n // 2, m) - 3
                events.append((max(0, dl), 4096,
                               lambda tag, m=m, n=n: emit_q_chunk(m, n, tag)))
        events.sort(key=lambda e: e[0])
        for dl, cyc, fn in events:
            F(cyc, fn, deadline=dl)

        # ================= attention units =================
        s_of = {}      # g -> s psum tile
        p_of = {}      # (u, j) -> p tile
        o_ps_of = {}   # u -> (o_a, o_b) psum accumulators
        osb_of = {}    # (i, c) -> normalized-O sbuf tile (fp8, x64)

        def emit_scores(g):
            u, j = divmod(g, TJ)
            i, hh = seq[u]
            if j == 0 and hh == 0 and i not in ot_tiles:
                ot_tiles[i] = otpool.tile([P, CP, IB], fp8, tag="ot",
                                          name="ot")
            quad, h4 = divmod(hh, 4)
            m = 2 * quad + h4 // 2
            r0 = 64 * (h4 % 2)
            s = psp.tile([P, IB], f32, tag="s", bufs=2, name="s")
            for iq in range(2):
                c0 = i * IB + iq * 512
                nc.tensor.matmul(
                    s[:, iq * 512:(iq + 1) * 512],
                    kt2[r0:r0 + 64, m, j * P:(j + 1) * P],
                    qt2[r0:r0 + 64, m, c0:c0 + 512],
                    start=True, stop=True)
            s_of[g] = s

        def emit_exp(g):
            u, j = divmod(g, TJ)
            s = s_of.pop(g)
            p = ppool.tile([P, IB], bf16, tag="p", name="p")
            if j in DVE_EXP_J:
                nc.vector.tensor_scalar(p[:].bitcast(i16), s[:],
                                        TRICK_A, TRICK_B, op0=MUL, op1=ADD)
            else:
                nc.scalar.activation(p[:], s[:], AF.Exp, scale=SCALE)
            p_of[(u, j)] = p

        def emit_pv(u, j):
            i, hh = seq[u]
            if j == 0:
                o_a = psp.tile([P, 512], f32, tag="o", bufs=1, name="oa")
                o_b = psp.tile([P, 512], f32, tag="ob", bufs=1, name="ob")
                o_ps_of[u] = (o_a, o_b)
            o_a, o_b = o_ps_of[u]
            p = p_of.pop((u, j))
            for t in range(8):
                dst = o_a[:, t * 65:t * 65 + 65] if t < 7 else o_b[:, 0:65]
                first = j == 0 and t in (0, 7)
                last = j == TJ - 1 and t in (6, 7)
                nc.tensor.matmul(dst, p[:, t * P:(t + 1) * P],
                                 vsb[:, j, hh, :],
                                 start=first, stop=last,
                                 skip_group_check=not (first or last))

        def emit_norm(u):
            """osb = O * (OSCALE / denom) in fp8 (x64 rides the rcp mul)."""
            i, hh = seq[u]
            c, hp = divmod(hh, 2)
            o_a, o_b = o_ps_of.pop(u)
            oa3 = o_a[:, 0:455].rearrange("p (t f) -> p t f", f=65)
            rcp = dpool.tile([P, 8], f32, tag="rcp", name="rcp")
            nc.vector.reciprocal(rcp[:, 0:7], oa3[:, :, 64])
            nc.vector.reciprocal(rcp[:, 7:8], o_b[:, 64:65])
            if hp == 0:
                osb_of[(i, c)] = ospool.tile([P, 8, 2, 64], fp8, tag="osb",
                                             name="osb")
            osb = osb_of[(i, c)]
            nc.vector.scalar_tensor_tensor(
                osb[:, 0:7, hp, :], oa3[:, :, 0:64], OSCALE,
                rcp[:, 0:7, None].broadcast_to([P, 7, 64]),
                op0=MUL, op1=MUL)
            nc.vector.scalar_tensor_tensor(
                osb[:, 7:8, hp, :], o_b[:, None, 0:64], OSCALE,
                rcp[:, 7:8, None].broadcast_to([P, 1, 64]),
                op0=MUL, op1=MUL)

        def emit_transposes(i, c, g):
            """O pair-block [qtok, 128feat] -> ot [128feat, qtok], fp8 PE
            transposes into an fp8 view of the psum bank; one 4-transpose
            group per call (g in {0,1}) to cap the per-step PE burst."""
            osb = osb_of[(i, c)]
            if g == 1:
                del osb_of[(i, c)]
            yslot = psp.tile([P, 512], f32, tag="y", bufs=2, name="tp")
            for tt in range(4):
                t = g * 4 + tt
                nc.tensor.matmul(
                    yslot[:, tt * 32:(tt + 1) * 32].bitcast(fp8),
                    osb[:, t, :, :].rearrange("p a b -> p (a b)"),
                    ident_f8[:], is_transpose=True,
                    start=(tt == 0), stop=(tt == 3),
                    skip_group_check=tt not in (0, 3))
            nc.scalar.activation(
                ot_tiles[i][:, c, g * 512:(g + 1) * 512],
                yslot[:, 0:128].bitcast(fp8), AF.Copy)

        CREDIT_PER_STEP = 680
        CREDIT_CAP = 3000
        PVLAG = 4
        credit = -2200    # delay the first credit pops past the prefix chain
        pending = {}
        tp_done = {0: 0, 1: 0}
        emit_scores(0)
        for g in range(NU * TJ + PVLAG + 4):
            if g < NU * TJ:
                emit_exp(g)
            flush_copies()   # last step's filler copies, behind the exp
            gp = g - PVLAG
            if 0 <= gp < NU * TJ:
                up, jp = divmod(gp, TJ)
                if jp == 0:
                    pass                      # deferred: paired with j1
                elif jp == 1:
                    emit_pv(up, 0)
                    emit_pv(up, 1)
                else:
                    emit_pv(up, jp)
            if g + 1 < NU * TJ:
                # scores next: early in the PE queue (right after the small
                # PV block) so exp(g+1) can overlap exp(g) on the other bank
                emit_scores(g + 1)
            if 0 <= gp < NU * TJ and gp % TJ == TJ - 1:
                up = gp // TJ
                iup, hup = seq[up]
                emit_norm(up)
                if hup % 2 == 1:
                    pending.setdefault(g + 5, []).append(
                        ("tp", iup, hup // 2, 0))
                    pending.setdefault(g + 6, []).append(
                        ("tp", iup, hup // 2, 1))
            for kind, a1, a2, a3 in pending.pop(g, []):
                emit_transposes(a1, a2, a3)
                if a3 == 1:
                    tp_done[a1] += 1
                    if tp_done[a1] == CP:
                        for t in range(8):
                            for o in range(2):
                                F(512, lambda tag, ii=a1, tt=t, oo=o:
                                  emit_proj(ii, tt, oo, tag))
            credit = min(credit + CREDIT_PER_STEP, CREDIT_CAP)
            credit = pump(credit, g)
        for gq in sorted(pending):
            for kind, a1, a2, a3 in pending[gq]:
                emit_transposes(a1, a2, a3)
                if a3 == 1:
                    tp_done[a1] += 1
                    if tp_done[a1] == CP:
                        for t in range(8):
                            for o in range(2):
                                F(512, lambda tag, ii=a1, tt=t, oo=o:
                                  emit_proj(ii, tt, oo, tag))

        # tail: drain remaining fillers at full rate (s banks free now)
        tags = ("s", "s", "y", "y")
        k = 0
        while fillers:
            _, _, fn = fillers.popleft()
            fn(tags[k % 4])
            flush_copies()
            k += 1
        flush_copies()

    nc.compile()
    return nc


def _get_nc():
    if "nc" not in _cached:
        _cached["nc"] = _build()
    return _cached["nc"]


def kernel(x, W_qkv, b_qkv, W_proj, b_proj):
    import ml_dtypes
    from concourse.bass_utils import run_bass_kernel_spmd

    bf16 = ml_dtypes.bfloat16
    fp8 = ml_dtypes.float8_e4m3
    x = np.asarray(x, dtype=np.float32)
    W_qkv = np.asarray(W_qkv, dtype=np.float32)
    b_qkv = np.asarray(b_qkv, dtype=np.float32)
    W_proj = np.asarray(W_proj, dtype=np.float32)
    b_proj = np.asarray(b_proj, dtype=np.float32)

    # feat permutation: output partition q of stationary chunk m holds
    # feature f = 256*(m//2) + 64*(2*(m%2) + q//64) + q%64
    mm, qq = np.meshgrid(np.arange(4), np.arange(P), indexing="ij")
    feat_idx = 256 * (mm // 2) + 64 * (2 * (mm % 2) + qq // 64) + qq % 64

    def pack_qk(w):      # [D, 512] -> [P, 4m, KC, 128q] bf16
        a = w.reshape(KC, P, DH).transpose(1, 0, 2)         # [p, k, f]
        a = a[..., feat_idx]                                # [p, k, m, q]
        return np.ascontiguousarray(a.transpose(0, 2, 1, 3)).astype(bf16)

    in_maps = []
    for core in range(NCORES):
        b, hg = divmod(core, HG)
        qs = slice(DH * hg, DH * (hg + 1))
        ks = slice(D + DH * hg, D + DH * (hg + 1))
        vs = slice(2 * D + DH * hg, 2 * D + DH * (hg + 1))
        xT = np.ascontiguousarray(x[b].T)                   # [D, N]
        bq = np.concatenate([
            b_qkv[qs][feat_idx].T,                          # [128, 4] q
            b_qkv[ks][feat_idx].T,                          # [128, 4] k
        ], axis=1).astype(np.float32)                       # [128, 8]

        in_maps.append({
            "xt": np.ascontiguousarray(
                xT.reshape(KC, P, N).transpose(1, 0, 2)).astype(bf16),
            "wqh": pack_qk(W_qkv[:, qs]),
            "wkh": pack_qk(W_qkv[:, ks]),
            "wvh": np.ascontiguousarray(
                W_qkv[:, vs].reshape(KC, P, HH * 64)
                .transpose(1, 0, 2)).astype(bf16),
            # W_proj x64 (fp8 denormal avoidance) in DR layout [p, i, ch, o]:
            # contraction feat = ch*256 + i*128 + p
            "wp8": np.ascontiguousarray(
                (W_proj[DH * hg:DH * (hg + 1), :] * OSCALE)
                .reshape(2, 2, P, D).transpose(2, 1, 0, 3)).astype(fp8),
            "bqh": np.ascontiguousarray(bq),
        })

    nc = _get_nc()
    res = run_bass_kernel_spmd(nc, in_maps, core_ids=list(range(NCORES)))
    beff = (b_proj.astype(np.float64)
            + b_qkv[2 * D:].astype(np.float64) @ W_proj.astype(np.float64)
            ).astype(np.float32)
    out = np.empty((B, N, D), dtype=np.float32)
    for b in range(B):
        out[b] = (res.results[2 * b]["y"].astype(np.float32)
                  + res.results[2 * b + 1]["y"].astype(np.float32) + beff)
    return out
